# revision 1
# baseline (speedup 1.0000x reference)
"""Neighbor aggregation (gnn message passing) Bass kernel for Trainium2.

out[b, i] = sum_{e: src[e]==i} w[e] * H[b, dst[e]]   (per batch b)

8 NeuronCores: core = 2*b + s handles batch b, src-half s (output rows
[s*25000, (s+1)*25000)).  Edges are partitioned host-side by dst-half into two
phases so gather indices fit int16 after rebasing.  Per 1024-token chunk:
SWDGE dma_gather (HBM H rows -> SBUF token-major), DVE broadcast multiply by
w, SWDGE dma_scatter_add (CCE f32) into a parity-split SBUF accumulator.

Hardware constraints found by probing:
 - SWDGE gather/scatter calls are limited to 1024 tokens (64 descriptors per
   DMA engine per packet); larger calls crash the device.
 - dma_scatter_add loses read-modify-write updates when the same destination
   row appears twice in close proximity within one call, so the host packs
   tokens into chunks with UNIQUE src per chunk (round-aligned bins) and pads
   with a junk accumulator row (>= 25000) whose updates are discarded.
"""

import os
import sys

sys.path.insert(0, "/opt/trn_rl_repo")

import numpy as np

import concourse.bacc as bacc
import concourse.mybir as mybir
import concourse.tile as tile
from concourse.bass_utils import run_bass_kernel_spmd

B, N, E, HS = 4, 50000, 800000, 64
NHALF = N // 2                  # 25000
C = 1024                        # tokens per chunk (hard HW limit per SWDGE call)
NGRP = 98                       # parity groups: accumulator covers idx < 25088
PAD_ROW = 25080                 # junk accumulator row for padding tokens

LAST_RESULT = {}


def build(nc, ch_per_phase, n_nodes, nhalf, c, ngrp, hs):
    f32 = mybir.dt.float32
    i16 = mybir.dt.int16

    h_d = nc.dram_tensor("h", [n_nodes, hs], f32, kind="ExternalInput")
    gidx_d = nc.dram_tensor(
        "gidx", [2, ch_per_phase, 128, c // 16], i16, kind="ExternalInput"
    )
    sidx_d = nc.dram_tensor(
        "sidx", [2, ch_per_phase, 128, c // 16], i16, kind="ExternalInput"
    )
    wl_d = nc.dram_tensor(
        "wl", [2, ch_per_phase, 128, c // 128], f32, kind="ExternalInput"
    )
    acc_d = nc.dram_tensor("acc", [2, 2, 128, ngrp, hs], f32, kind="ExternalOutput")

    with tile.TileContext(nc) as tc:
        with tc.tile_pool(name="accp", bufs=1) as accp, \
             tc.tile_pool(name="work", bufs=4) as wp:
            accs = []
            for pr in range(2):
                a0 = accp.tile([128, ngrp, hs], f32, tag=f"acc{pr}0")
                a1 = accp.tile([128, ngrp, hs], f32, tag=f"acc{pr}1")
                nc.vector.memset(a0[:], 0.0)
                nc.vector.memset(a1[:], 0.0)
                accs.append((a0, a1))

            for phase in range(2):
                h_slice = h_d[:][phase * nhalf:(phase + 1) * nhalf, :]
                for k in range(ch_per_phase):
                    gi = wp.tile([128, c // 16], i16, tag="gi")
                    si = wp.tile([128, c // 16], i16, tag="si")
                    wt = wp.tile([128, c // 128], f32, tag="wt")
                    nc.sync.dma_start(gi[:], gidx_d[phase, k])
                    nc.sync.dma_start(si[:], sidx_d[phase, k])
                    nc.sync.dma_start(wt[:], wl_d[phase, k])

                    msgs = wp.tile([128, c // 128, hs], f32, tag="msgs")
                    nc.gpsimd.dma_gather(
                        out_ap=msgs[:],
                        in_ap=h_slice,
                        idxs_ap=gi[:],
                        num_idxs=c,
                        num_idxs_reg=c,
                        elem_size=hs,
                    )
                    nc.vector.tensor_tensor(
                        out=msgs[:],
                        in0=msgs[:],
                        in1=wt[:].unsqueeze(2).broadcast_to([128, c // 128, hs]),
                        op=mybir.AluOpType.mult,
                    )
                    a0, a1 = accs[k % 2]
                    nc.gpsimd.dma_scatter_add(
                        out_ap=a0[:],
                        in_ap=msgs[:],
                        idxs_ap=si[:],
                        num_idxs=c,
                        num_idxs_reg=c,
                        elem_size=hs,
                        sbuf_tokens_per_rank=128,
                        parity_reg=0,
                        out_ap_other=a1[:],
                    )

            for pr in range(2):
                nc.sync.dma_start(acc_d[pr, 0], accs[pr][0][:])
                nc.sync.dma_start(acc_d[pr, 1], accs[pr][1][:])
    return nc


_COMPILED = {}


def _get_compiled(ch_per_phase):
    if ch_per_phase not in _COMPILED:
        nc = bacc.Bacc("TRN2", target_bir_lowering=False, debug=False)
        build(nc, ch_per_phase, N, NHALF, C, NGRP, HS)
        nc.compile()
        _COMPILED[ch_per_phase] = nc
    return _COMPILED[ch_per_phase]


def _wrap16(idx, ch, c):
    a = idx.reshape(ch, c // 16, 16).transpose(0, 2, 1).astype(np.int16)
    return np.ascontiguousarray(np.tile(a, (1, 8, 1)))


def _round_pack(srcs, dsts, ws, cap):
    """Order tokens so equal src never share a 1024-token chunk: tokens get a
    within-src rank (round); each round starts at a fresh chunk boundary.
    Returns (g, s, w) arrays of length n_chunks*cap with pads."""
    order = np.argsort(srcs, kind="stable")
    ss = srcs[order]
    # within-group rank
    n = ss.shape[0]
    if n == 0:
        return (np.zeros(cap, np.int64), np.full(cap, PAD_ROW, np.int64),
                np.zeros(cap, np.float32), 1)
    first = np.r_[True, ss[1:] != ss[:-1]]
    gstart = np.flatnonzero(first)
    rank = np.arange(n) - np.repeat(gstart, np.diff(np.r_[gstart, n]))
    # order by (rank, position) stable -> rounds contiguous
    order2 = np.argsort(rank, kind="stable")
    rank_s = rank[order2]
    tok = order[order2]
    nr = np.bincount(rank_s)
    chunks_per_round = -(-nr // cap)
    starts = np.concatenate([[0], np.cumsum(chunks_per_round[:-1] * cap)])
    total_chunks = int(chunks_per_round.sum())
    pos = starts[rank_s] + (np.arange(n) - np.repeat(
        np.concatenate([[0], np.cumsum(nr[:-1])]), nr))
    cap_total = total_chunks * cap
    g = np.zeros(cap_total, np.int64)
    s = np.full(cap_total, PAD_ROW, np.int64)
    w = np.zeros(cap_total, np.float32)
    g[pos] = dsts[tok]
    s[pos] = srcs[tok]
    w[pos] = ws[tok]
    return g, s, w, total_chunks


def _prep_core(src, dst, w, s, ch):
    sel = (src >= NHALF) == bool(s)
    srcs = src[sel] - s * NHALF
    dsts = dst[sel]
    ws = w[sel]

    cap = ch * C
    g_all = np.zeros((2, cap), np.int64)
    s_all = np.full((2, cap), PAD_ROW, np.int64)
    w_all = np.zeros((2, cap), np.float32)
    for phase in range(2):
        pm = (dsts >= NHALF) == bool(phase)
        g, sarr, warr, nch = _round_pack(
            srcs[pm], dsts[pm] - phase * NHALF, ws[pm], C)
        assert nch <= ch, f"phase overflow: {nch} > {ch}"
        g_all[phase, :nch * C] = g
        s_all[phase, :nch * C] = sarr
        w_all[phase, :nch * C] = warr

    out = {}
    out["gidx"] = np.stack([_wrap16(g_all[p], ch, C) for p in range(2)])
    out["sidx"] = np.stack([_wrap16(s_all[p], ch, C) for p in range(2)])
    out["wl"] = np.ascontiguousarray(
        w_all.reshape(2, ch, C // 128, 128).transpose(0, 1, 3, 2)
    )
    return out


def _needed_chunks(src, dst, w):
    """Chunks per phase = sum over rounds r of ceil(#nodes-with-count>r / C)."""
    worst = 1
    for b in range(B):
        for s in range(2):
            sel = (src[b] >= NHALF) == bool(s)
            srcs = src[b][sel] - s * NHALF
            dsts = dst[b][sel]
            for phase in range(2):
                pm = (dsts >= NHALF) == bool(phase)
                ss = srcs[pm]
                cnts = np.bincount(ss, minlength=1)
                mx = int(cnts.max()) if cnts.size else 0
                rounds = np.array([(cnts > r).sum() for r in range(mx)])
                nch = int(np.sum(-(-rounds // C))) if mx else 1
                worst = max(worst, nch)
    return worst


def kernel(**inputs):
    H = np.ascontiguousarray(np.asarray(inputs["H"], np.float32))
    w = np.asarray(inputs["edge_w"], np.float32)
    src = np.asarray(inputs["edge_src"], np.int64)
    dst = np.asarray(inputs["edge_dst"], np.int64)

    ch = _needed_chunks(src, dst, w)
    nc = _get_compiled(ch)

    in_maps = []
    for core in range(8):
        b, s = core // 2, core % 2
        m = _prep_core(src[b], dst[b], w[b], s, ch)
        m["h"] = H[b]
        in_maps.append(m)

    trace = bool(int(os.environ.get("GNN_TRACE", "0")))
    res = run_bass_kernel_spmd(nc, in_maps, list(range(8)), trace=trace)
    LAST_RESULT["exec_time_ns"] = res.exec_time_ns
    LAST_RESULT["res"] = res

    out = np.empty((B, N, HS), np.float32)
    for core in range(8):
        b, s = core // 2, core % 2
        acc = res.results[core]["acc"].sum(axis=0)  # [2, 128, NGRP, HS]
        rows = acc.transpose(2, 0, 1, 3).reshape(-1, HS)[:NHALF]
        out[b, s * NHALF:(s + 1) * NHALF] = rows
    return out

